# revision 45
# baseline (speedup 1.0000x reference)
"""Trainium2 Bass kernel for the dense GNN message-passing step.

Computation (N=16384, NUM_IN=1024, NUM_OUT=256):
    states = zeros(N); states[input_indices] = input_values
    total  = states @ W + biases                      # GEMV over [N, N] f32
    out    = act_select(total)[output_indices]        # 0=id, 1=relu, 2=softsign

Strategy:
  * `states` is zero outside the (<=1024) positions named by input_indices,
    so only those rows of W contribute to the GEMV. The host gathers the
    live rows and the device contracts over a padded K=1024 instead of
    16384 -> 16x less HBM traffic.
  * W is sharded column-wise across the 8 cores (tensor parallel): each
    core computes its 2048 outputs = GEMV slice + bias + per-neuron
    activation select; the host concatenates and gathers output_indices.
  * W is stored as fp8 e3m4 scaled by 64 (1 B/element; rel err ~7e-3 vs
    the 2e-2 gate); x is folded by 1/64 in fp16 (exact power-of-2), so
    x' W is computed with no device-side descale. 2 MB/core total HBM.
  * The 2 x 1MB W-chunk DMAs are issued back-to-back up front on the sync
    HWDGE queue (FIFO, 8KB/partition descriptors ~ line rate); chunk h
    covers chunk-pair h so pair 0's matmuls start ~1 chunk in.
  * x is stationary: for (kc, j) a [128, 2] fp16 block with x chunk kc in
    column j, zero in the other. Each matmul accumulates a [2, 512] PSUM
    tile (row j real, +0 elsewhere), so pair {2h, 2h+1} lands on
    contiguous partitions {0,1} of bank h (DVE cannot read strided
    partitions; PE output base partition is restricted to {0,32,64}).
  * ~3.4us of dummy warm-up matmuls fill the dead preamble window so the
    HAM un-throttles the PE clock (1.2 -> 2.4 GHz) before W arrives; warm
    matmuls pipeline at ~216ns vs 634ns cold.
  * Select-free epilogue (7 ops/pair, no copy_predicated, no masks):
        t  = P + b
        tm = t * m2f          (m2f = 1 where softsign else 0)
        rc = 1/(1 + |tm|)     (ACT Abs + ACT Copy(bias=1) + DVE recip)
        y  = max(t * rc, L)   (L = 0 where relu else -3e38)
    For softsign neurons rc = 1/(1+|t|) and L=-inf -> t/(1+|t|); for relu
    neurons rc = 1 and L=0 -> max(t,0); else rc=1, L=-inf -> t.
"""

import numpy as np
from contextlib import ExitStack

import concourse.bacc as bacc
import concourse.tile as tile
from concourse import mybir
from concourse.bass_utils import run_bass_kernel_spmd

N_CORES = 8
K = 1024                 # padded contraction size (live rows)
KC = K // 128            # 8 k-chunks
NPC = 16384 // N_CORES   # 2048 output columns per core
NCH = NPC // 512         # 4 column chunks of 512
WS = 64.0                # fp8 weight scale (power of 2; x carries 1/WS)
F32 = mybir.dt.float32
F16 = mybir.dt.float16
F8 = mybir.dt.float8e3
U8 = mybir.dt.uint8

_BUILT = None            # cached nc so repeat calls reuse the compiled module
LAST_RESULTS = None      # BassKernelResults of the most recent run (for test.py)


def _build_bass():
    nc = bacc.Bacc(
        "TRN2", target_bir_lowering=False, debug=False, num_devices=N_CORES
    )
    # Pair 0 ships as two 512KB quarter-chunks (j=0, j=1) so its matmuls
    # (and epilogue) start earlier; pair 1 as one 1MB chunk.
    # w0[j]: [p, kc*512 + c] = W[kc*128+p, 512j + c] (x64, e3m4).
    # w1: [p, (j*KC+kc)*512 + c] = W[kc*128+p, 1024 + 512j + c].
    w0 = nc.dram_tensor("w0", [2, 128, KC * 512], F8, kind="ExternalInput").ap()
    w1 = nc.dram_tensor("w1", [2, 128, KC * 512], F8, kind="ExternalInput").ap()
    # xs cols 0..63: stationary x blocks; cols 64..65: a [2,2] identity
    # for the K=2 bias matmul.
    xs = nc.dram_tensor("xs", [128, KC * 4 + 2], F16, kind="ExternalInput").ap()
    # bias rows packed [row(2), half(2)*512] f16 — moving operand of the
    # bias matmul.
    bh = nc.dram_tensor("bh", [2, 2 * 512], F16, kind="ExternalInput").ap()
    # aux packed [row(2), L|m2f x half(2)*512] f32: row j, col-block h
    # holds chunk 2h+j's values.
    aux = nc.dram_tensor("aux", [2, 2 * 1024], F32, kind="ExternalInput").ap()
    o = nc.dram_tensor("o", [NCH, 512], F32, kind="ExternalOutput").ap()

    with tile.TileContext(nc) as tc:
        with ExitStack() as ctx:
            small = ctx.enter_context(tc.tile_pool(name="small", bufs=1))
            wpool = ctx.enter_context(tc.tile_pool(name="wp", bufs=2))
            ppool = ctx.enter_context(tc.tile_pool(name="pp", bufs=1, space="PSUM"))
            scr = ctx.enter_context(tc.tile_pool(name="scr", bufs=1))

            # W as four 512KB quarter-chunks (h, j), FIFO on the sync HWDGE
            # queue — each quarter releases 8 matmuls ~1.8us apart.
            wts = {}
            for h, wsrc in ((0, w0), (1, w1)):
                for j in range(2):
                    wt = wpool.tile(
                        [128, KC * 512], F8, tag="wq", name=f"wq{h}{j}"
                    )
                    nc.sync.dma_start(wt[:], wsrc[j])
                    wts[(h, j)] = wt

            def wslice(half, j, kc):
                return wts[(half, j)][:, kc * 512 : (kc + 1) * 512]

            # Small tensors on the scalar HWDGE queue: bh first (the bias
            # matmul opens each PSUM group), then xs, then aux.
            bh_t = small.tile([2, 2 * 512], F16, tag="bh")
            nc.scalar.dma_start(bh_t[:], bh[:])
            xs_t = small.tile([128, KC * 4 + 2], F16, tag="xs")
            nc.scalar.dma_start(xs_t[:], xs[:])
            aux_t = small.tile([2, 2 * 1024], F32, tag="aux")
            nc.scalar.dma_start(aux_t[:], aux[:])
            l_t = aux_t[:, 0:1024]
            m2_t = aux_t[:, 1024:2048]

            # PE warm-up: ~3.4us of dummy matmuls during the dead preamble
            # window so the HAM un-throttles the PE clock before W arrives.
            wu = scr.tile([128, 448], F16, tag="wu")
            nc.gpsimd.memset(wu[:], 0.0)
            pw = ppool.tile([128, 512], F32, tag="pw")
            for _ in range(8):
                nc.tensor.matmul(
                    pw[0:1, 0:448], wu[:, 0:1], wu[:], start=True, stop=True
                )

            # Chunk pair {2h, 2h+1} -> PSUM bank h rows {0,1}: one
            # accumulation group per bank = K=2 bias matmul (identity
            # stationary x [2,512] bias rows, start=True) + 16 x-matmuls.
            # After the group closes, PSUM holds t = x'W + b directly.
            eye2 = xs_t[0:2, KC * 4 : KC * 4 + 2]
            pt0 = ppool.tile([128, 512], F32, tag="p0")
            pt1 = ppool.tile([128, 512], F32, tag="p1")
            pts = [pt0, pt1]
            for half in range(2):
                cs = slice(half * 512, (half + 1) * 512)
                pt = pts[half]
                nc.tensor.matmul(
                    pt[0:2, :], eye2, bh_t[:, cs], start=True, stop=False
                )
                for j in range(2):
                    for kc in range(KC):
                        blk = (kc * 2 + j) * 2
                        nc.tensor.matmul(
                            pt[0:2, :],
                            xs_t[:, blk : blk + 2],
                            wslice(half, j, kc),
                            start=False,
                            stop=(j == 1 and kc == KC - 1),
                        )

            # Epilogue per pair on contiguous [2,512], reading t from PSUM;
            # pair 0 overlaps pair 1's matmuls.
            for half in range(2):
                cs = slice(half * 512, (half + 1) * 512)
                p2 = pts[half][0:2, :]
                tm = scr.tile([2, 512], F32, tag=f"tm{half}", name=f"tm{half}")
                at = scr.tile([2, 512], F32, tag=f"at{half}", name=f"at{half}")
                a1 = scr.tile([2, 512], F32, tag=f"a1{half}", name=f"a1{half}")
                rc = scr.tile([2, 512], F32, tag=f"rc{half}", name=f"rc{half}")
                sf = scr.tile([2, 512], F32, tag=f"sf{half}", name=f"sf{half}")
                nc.vector.tensor_mul(tm[:], p2, m2_t[:, cs])   # t * m2f
                nc.scalar.activation(                          # |t*m2f|
                    at[:], tm[:], mybir.ActivationFunctionType.Abs
                )
                nc.scalar.activation(                          # 1 + |t*m2f|
                    a1[:], at[:], mybir.ActivationFunctionType.Copy, bias=1.0
                )
                nc.vector.reciprocal_approx_fast(rc[:], a1[:])
                nc.vector.tensor_mul(sf[:], p2, rc[:])         # t * rc
                nc.vector.tensor_max(sf[:], sf[:], l_t[:, cs])  # relu select
                nc.sync.dma_start(o[2 * half : 2 * half + 2], sf[:])

    nc.compile()
    return nc


def kernel(**inputs) -> np.ndarray:
    global _BUILT, LAST_RESULTS

    import ml_dtypes

    iv = np.asarray(inputs["input_values"], dtype=np.float32)
    W = np.asarray(inputs["weight_matrix"], dtype=np.float32)
    bias = np.asarray(inputs["biases"], dtype=np.float32)
    act = np.asarray(inputs["act_ids"])
    iidx = np.asarray(inputs["input_indices"]).astype(np.int64)
    oidx = np.asarray(inputs["output_indices"]).astype(np.int64)

    n = W.shape[0]
    # Dense neuron-state vector (duplicate indices: last write wins, matching
    # jax's .at[].set) and its index support.
    states = np.zeros(n, np.float32)
    states[iidx] = iv
    live = np.zeros(n, dtype=bool)
    live[iidx] = True
    support = np.flatnonzero(live)
    assert support.size <= K, "more than K live rows not supported"
    rows = np.zeros(K, np.int64)          # pad with row 0 (x=0 there => no-op)
    rows[: support.size] = support
    xvec = np.zeros(K, np.float32)
    xvec[: support.size] = states[support]

    Wq = (W[rows] * WS).astype(ml_dtypes.float8_e3m4)   # [K, n] live rows
    xh = (xvec / WS).astype(np.float16)
    xc = xh.reshape(KC, 128).T            # [128, KC]
    # Stationary blocks [128, (kc*2+j)*2 + m]: x chunk kc in column m==j;
    # trailing [2,2] identity for the bias matmul.
    xs_t = np.zeros((128, KC * 4 + 2), np.float16)
    for kc in range(KC):
        for j in range(2):
            xs_t[:, (kc * 2 + j) * 2 + j] = xc[:, kc]
    xs_t[0, KC * 4] = 1.0
    xs_t[1, KC * 4 + 1] = 1.0

    lsel = np.where(act == 1, 0.0, -3.0e38).astype(np.float32)
    m2f = (act == 2).astype(np.float32)

    in_maps = []
    for c in range(N_CORES):
        sl = slice(c * NPC, (c + 1) * NPC)
        # [kc, p, h, j, c] -> [h, p, j, kc, c]
        wc = Wq[:, sl].reshape(KC, 128, 2, 2, 512).transpose(2, 1, 3, 0, 4)

        def pack2(a):
            # [NCH*512] -> [row(2), half(2)*512]: packed[r, 512h+j] = chunk
            # (2h+r) col j, matching the aux SBUF layout.
            return a.reshape(2, 2, 512).transpose(1, 0, 2).reshape(2, 1024)

        in_maps.append(
            {
                "w0": np.ascontiguousarray(
                    wc[0].transpose(1, 0, 2, 3)     # [j, p, kc, c]
                ).reshape(2, 128, KC * 512),
                "w1": np.ascontiguousarray(
                    wc[1].transpose(1, 0, 2, 3)
                ).reshape(2, 128, KC * 512),
                "xs": xs_t,
                "bh": np.ascontiguousarray(
                    pack2(bias[sl].astype(np.float16))
                ),
                "aux": np.ascontiguousarray(
                    np.concatenate([pack2(lsel[sl]), pack2(m2f[sl])], axis=1)
                ),
            }
        )

    if _BUILT is None:
        _BUILT = _build_bass()
    LAST_RESULTS = run_bass_kernel_spmd(
        _BUILT, in_maps, core_ids=list(range(N_CORES))
    )
    full = np.concatenate(
        [LAST_RESULTS.results[c]["o"].reshape(-1) for c in range(N_CORES)]
    )
    return full[oidx].astype(np.float32)


# revision 48
# speedup vs baseline: 1.1346x; 1.1346x over previous
"""Trainium2 Bass kernel for the dense GNN message-passing step.

Computation (N=16384, NUM_IN=1024, NUM_OUT=256):
    states = zeros(N); states[input_indices] = input_values
    total  = states @ W + biases                      # GEMV over [N, N] f32
    out    = act_select(total)[output_indices]        # 0=id, 1=relu, 2=softsign

Strategy:
  * `states` is zero outside the (<=1024) positions named by input_indices,
    so only those rows of W contribute to the GEMV. The host gathers the
    live rows and the device contracts over a padded K=1024 instead of
    16384 -> 16x less HBM traffic.
  * W is sharded column-wise across the 8 cores (tensor parallel): each
    core computes its 2048 outputs = GEMV slice + bias + per-neuron
    activation select; the host concatenates and gathers output_indices.
  * W is stored as fp8 e3m4 scaled by 64 (1 B/element; rel err ~7e-3 vs
    the 2e-2 gate); x is folded by 1/64 in fp16 (exact power-of-2), so
    x' W is computed with no device-side descale. 2 MB/core total HBM.
  * The 2 x 1MB W-chunk DMAs are issued back-to-back up front on the sync
    HWDGE queue (FIFO, 8KB/partition descriptors ~ line rate); chunk h
    covers chunk-pair h so pair 0's matmuls start ~1 chunk in.
  * x is stationary: for (kc, j) a [128, 2] fp16 block with x chunk kc in
    column j, zero in the other. Each matmul accumulates a [2, 512] PSUM
    tile (row j real, +0 elsewhere), so pair {2h, 2h+1} lands on
    contiguous partitions {0,1} of bank h (DVE cannot read strided
    partitions; PE output base partition is restricted to {0,32,64}).
  * ~3.4us of dummy warm-up matmuls fill the dead preamble window so the
    HAM un-throttles the PE clock (1.2 -> 2.4 GHz) before W arrives; warm
    matmuls pipeline at ~216ns vs 634ns cold.
  * Select-free epilogue (7 ops/pair, no copy_predicated, no masks):
        t  = P + b
        tm = t * m2f          (m2f = 1 where softsign else 0)
        rc = 1/(1 + |tm|)     (ACT Abs + ACT Copy(bias=1) + DVE recip)
        y  = max(t * rc, L)   (L = 0 where relu else -3e38)
    For softsign neurons rc = 1/(1+|t|) and L=-inf -> t/(1+|t|); for relu
    neurons rc = 1 and L=0 -> max(t,0); else rc=1, L=-inf -> t.
"""

import numpy as np
from contextlib import ExitStack

import concourse.bacc as bacc
import concourse.tile as tile
from concourse import mybir
from concourse.bass_utils import run_bass_kernel_spmd

N_CORES = 8
K = 1024                 # padded contraction size (live rows)
KC = K // 128            # 8 k-chunks
NPC = 16384 // N_CORES   # 2048 output columns per core
NCH = NPC // 512         # 4 column chunks of 512
WS = 64.0                # fp8 weight scale (power of 2; x carries 1/WS)
F32 = mybir.dt.float32
F16 = mybir.dt.float16
F8 = mybir.dt.float8e3
U8 = mybir.dt.uint8

_BUILT = None            # cached nc so repeat calls reuse the compiled module
LAST_RESULTS = None      # BassKernelResults of the most recent run (for test.py)


def _build_bass():
    nc = bacc.Bacc(
        "TRN2", target_bir_lowering=False, debug=False, num_devices=N_CORES
    )
    # Pair 0 ships as two 512KB quarter-chunks (j=0, j=1) so its matmuls
    # (and epilogue) start earlier; pair 1 as one 1MB chunk.
    # w0[j]: [p, kc*512 + c] = W[kc*128+p, 512j + c] (x64, e3m4).
    # w1: [p, (j*KC+kc)*512 + c] = W[kc*128+p, 1024 + 512j + c].
    w0 = nc.dram_tensor("w0", [2, 128, KC * 512], F8, kind="ExternalInput").ap()
    w1 = nc.dram_tensor("w1", [2, 128, KC * 512], F8, kind="ExternalInput").ap()
    # xs cols 0..63: stationary x blocks; cols 64..65: a [2,2] identity
    # for the K=2 bias matmul.
    xs = nc.dram_tensor("xs", [128, KC * 4 + 2], F16, kind="ExternalInput").ap()
    # bias rows packed [row(2), half(2)*512] f16 — moving operand of the
    # bias matmul.
    bh = nc.dram_tensor("bh", [2, 2 * 512], F16, kind="ExternalInput").ap()
    # aux packed [row(2), L|m2f x half(2)*512] f32: row j, col-block h
    # holds chunk 2h+j's values.
    aux = nc.dram_tensor("aux", [2, 2 * 1024], F32, kind="ExternalInput").ap()
    o = nc.dram_tensor("o", [NCH, 512], F32, kind="ExternalOutput").ap()

    with tile.TileContext(nc) as tc:
        with ExitStack() as ctx:
            small = ctx.enter_context(tc.tile_pool(name="small", bufs=1))
            wpool = ctx.enter_context(tc.tile_pool(name="wp", bufs=4))
            ppool = ctx.enter_context(tc.tile_pool(name="pp", bufs=1, space="PSUM"))
            scr = ctx.enter_context(tc.tile_pool(name="scr", bufs=1))

            # xs first on the sync queue (16KB; it gates the first matmul
            # and the scalar queue starts late behind the ACT table load),
            # then W as four 512KB quarter-chunks (h, j), FIFO — each
            # quarter releases 8 matmuls ~1.4us apart.
            xs_t = small.tile([128, KC * 4 + 2], F16, tag="xs")
            nc.sync.dma_start(xs_t[:], xs[:])
            wts = {}
            for h, wsrc in ((0, w0), (1, w1)):
                for j in range(2):
                    wt = wpool.tile(
                        [128, KC * 512], F8, tag="wq", name=f"wq{h}{j}"
                    )
                    nc.sync.dma_start(wt[:], wsrc[j])
                    wts[(h, j)] = wt

            def wslice(half, j, kc):
                return wts[(half, j)][:, kc * 512 : (kc + 1) * 512]

            # Small tensors on the scalar HWDGE queue: bh first (the bias
            # matmul opens each PSUM group), then aux.
            bh_t = small.tile([2, 2 * 512], F16, tag="bh")
            nc.scalar.dma_start(bh_t[:], bh[:])
            aux_t = small.tile([2, 2 * 1024], F32, tag="aux")
            nc.scalar.dma_start(aux_t[:], aux[:])
            l_t = aux_t[:, 0:1024]
            m2_t = aux_t[:, 1024:2048]

            # PE warm-up: ~3.4us of dummy matmuls during the dead preamble
            # window so the HAM un-throttles the PE clock before W arrives.
            wu = scr.tile([128, 448], F16, tag="wu")
            nc.gpsimd.memset(wu[:], 0.0)
            pw = ppool.tile([128, 512], F32, tag="pw")
            for _ in range(8):
                nc.tensor.matmul(
                    pw[0:1, 0:448], wu[:, 0:1], wu[:], start=True, stop=True
                )

            # Chunk pair {2h, 2h+1} -> PSUM bank h rows {0,1}: one
            # accumulation group per bank = K=2 bias matmul (identity
            # stationary x [2,512] bias rows, start=True) + 16 x-matmuls.
            # After the group closes, PSUM holds t = x'W + b directly.
            eye2 = xs_t[0:2, KC * 4 : KC * 4 + 2]
            pt0 = ppool.tile([128, 512], F32, tag="p0")
            pt1 = ppool.tile([128, 512], F32, tag="p1")
            pts = [pt0, pt1]
            for half in range(2):
                cs = slice(half * 512, (half + 1) * 512)
                pt = pts[half]
                nc.tensor.matmul(
                    pt[0:2, :], eye2, bh_t[:, cs], start=True, stop=False
                )
                for j in range(2):
                    for kc in range(KC):
                        blk = (kc * 2 + j) * 2
                        nc.tensor.matmul(
                            pt[0:2, :],
                            xs_t[:, blk : blk + 2],
                            wslice(half, j, kc),
                            start=False,
                            stop=(j == 1 and kc == KC - 1),
                        )

            # Epilogue per pair on contiguous [2,512], reading t from PSUM;
            # pair 0 overlaps pair 1's matmuls.
            for half in range(2):
                cs = slice(half * 512, (half + 1) * 512)
                p2 = pts[half][0:2, :]
                tm = scr.tile([2, 512], F32, tag=f"tm{half}", name=f"tm{half}")
                at = scr.tile([2, 512], F32, tag=f"at{half}", name=f"at{half}")
                a1 = scr.tile([2, 512], F32, tag=f"a1{half}", name=f"a1{half}")
                rc = scr.tile([2, 512], F32, tag=f"rc{half}", name=f"rc{half}")
                sf = scr.tile([2, 512], F32, tag=f"sf{half}", name=f"sf{half}")
                nc.vector.tensor_mul(tm[:], p2, m2_t[:, cs])   # t * m2f
                nc.scalar.activation(                          # |t*m2f|
                    at[:], tm[:], mybir.ActivationFunctionType.Abs
                )
                nc.scalar.activation(                          # 1 + |t*m2f|
                    a1[:], at[:], mybir.ActivationFunctionType.Copy, bias=1.0
                )
                nc.vector.reciprocal_approx_fast(rc[:], a1[:])
                nc.vector.tensor_mul(sf[:], p2, rc[:])         # t * rc
                nc.vector.tensor_max(sf[:], sf[:], l_t[:, cs])  # relu select
                nc.sync.dma_start(o[2 * half : 2 * half + 2], sf[:])

    nc.compile()
    return nc


def kernel(**inputs) -> np.ndarray:
    global _BUILT, LAST_RESULTS

    import ml_dtypes

    iv = np.asarray(inputs["input_values"], dtype=np.float32)
    W = np.asarray(inputs["weight_matrix"], dtype=np.float32)
    bias = np.asarray(inputs["biases"], dtype=np.float32)
    act = np.asarray(inputs["act_ids"])
    iidx = np.asarray(inputs["input_indices"]).astype(np.int64)
    oidx = np.asarray(inputs["output_indices"]).astype(np.int64)

    n = W.shape[0]
    # Dense neuron-state vector (duplicate indices: last write wins, matching
    # jax's .at[].set) and its index support.
    states = np.zeros(n, np.float32)
    states[iidx] = iv
    live = np.zeros(n, dtype=bool)
    live[iidx] = True
    support = np.flatnonzero(live)
    assert support.size <= K, "more than K live rows not supported"
    rows = np.zeros(K, np.int64)          # pad with row 0 (x=0 there => no-op)
    rows[: support.size] = support
    xvec = np.zeros(K, np.float32)
    xvec[: support.size] = states[support]

    Wq = (W[rows] * WS).astype(ml_dtypes.float8_e3m4)   # [K, n] live rows
    xh = (xvec / WS).astype(np.float16)
    xc = xh.reshape(KC, 128).T            # [128, KC]
    # Stationary blocks [128, (kc*2+j)*2 + m]: x chunk kc in column m==j;
    # trailing [2,2] identity for the bias matmul.
    xs_t = np.zeros((128, KC * 4 + 2), np.float16)
    for kc in range(KC):
        for j in range(2):
            xs_t[:, (kc * 2 + j) * 2 + j] = xc[:, kc]
    xs_t[0, KC * 4] = 1.0
    xs_t[1, KC * 4 + 1] = 1.0

    lsel = np.where(act == 1, 0.0, -3.0e38).astype(np.float32)
    m2f = (act == 2).astype(np.float32)

    in_maps = []
    for c in range(N_CORES):
        sl = slice(c * NPC, (c + 1) * NPC)
        # [kc, p, h, j, c] -> [h, p, j, kc, c]
        wc = Wq[:, sl].reshape(KC, 128, 2, 2, 512).transpose(2, 1, 3, 0, 4)

        def pack2(a):
            # [NCH*512] -> [row(2), half(2)*512]: packed[r, 512h+j] = chunk
            # (2h+r) col j, matching the aux SBUF layout.
            return a.reshape(2, 2, 512).transpose(1, 0, 2).reshape(2, 1024)

        in_maps.append(
            {
                "w0": np.ascontiguousarray(
                    wc[0].transpose(1, 0, 2, 3)     # [j, p, kc, c]
                ).reshape(2, 128, KC * 512),
                "w1": np.ascontiguousarray(
                    wc[1].transpose(1, 0, 2, 3)
                ).reshape(2, 128, KC * 512),
                "xs": xs_t,
                "bh": np.ascontiguousarray(
                    pack2(bias[sl].astype(np.float16))
                ),
                "aux": np.ascontiguousarray(
                    np.concatenate([pack2(lsel[sl]), pack2(m2f[sl])], axis=1)
                ),
            }
        )

    if _BUILT is None:
        _BUILT = _build_bass()
    LAST_RESULTS = run_bass_kernel_spmd(
        _BUILT, in_maps, core_ids=list(range(N_CORES))
    )
    full = np.concatenate(
        [LAST_RESULTS.results[c]["o"].reshape(-1) for c in range(N_CORES)]
    )
    return full[oidx].astype(np.float32)


# revision 52
# speedup vs baseline: 1.1404x; 1.0052x over previous
"""Trainium2 Bass kernel for the dense GNN message-passing step.

Computation (N=16384, NUM_IN=1024, NUM_OUT=256):
    states = zeros(N); states[input_indices] = input_values
    total  = states @ W + biases                      # GEMV over [N, N] f32
    out    = act_select(total)[output_indices]        # 0=id, 1=relu, 2=softsign

Strategy:
  * `states` is zero outside the (<=1024) positions named by input_indices,
    so only those rows of W contribute to the GEMV. The host gathers the
    live rows and the device contracts over a padded K=1024 instead of
    16384 -> 16x less HBM traffic.
  * W is sharded column-wise across the 8 cores (tensor parallel): each
    core computes its 2048 outputs = GEMV slice + bias + per-neuron
    activation select; the host concatenates and gathers output_indices.
  * W is stored as fp8 e3m4 scaled by 64 (1 B/element; rel err ~7e-3 vs
    the 2e-2 gate); x is folded by 1/64 in fp16 (exact power-of-2), so
    x' W is computed with no device-side descale. 2 MB/core total HBM.
  * The 2 x 1MB W-chunk DMAs are issued back-to-back up front on the sync
    HWDGE queue (FIFO, 8KB/partition descriptors ~ line rate); chunk h
    covers chunk-pair h so pair 0's matmuls start ~1 chunk in.
  * x is stationary: for (kc, j) a [128, 2] fp16 block with x chunk kc in
    column j, zero in the other. Each matmul accumulates a [2, 512] PSUM
    tile (row j real, +0 elsewhere), so pair {2h, 2h+1} lands on
    contiguous partitions {0,1} of bank h (DVE cannot read strided
    partitions; PE output base partition is restricted to {0,32,64}).
  * ~3.4us of dummy warm-up matmuls fill the dead preamble window so the
    HAM un-throttles the PE clock (1.2 -> 2.4 GHz) before W arrives; warm
    matmuls pipeline at ~216ns vs 634ns cold.
  * Select-free epilogue (7 ops/pair, no copy_predicated, no masks):
        t  = P + b
        tm = t * m2f          (m2f = 1 where softsign else 0)
        rc = 1/(1 + |tm|)     (ACT Abs + ACT Copy(bias=1) + DVE recip)
        y  = max(t * rc, L)   (L = 0 where relu else -3e38)
    For softsign neurons rc = 1/(1+|t|) and L=-inf -> t/(1+|t|); for relu
    neurons rc = 1 and L=0 -> max(t,0); else rc=1, L=-inf -> t.
"""

import numpy as np
from contextlib import ExitStack

import concourse.bacc as bacc
import concourse.tile as tile
from concourse import mybir
from concourse.bass_utils import run_bass_kernel_spmd

N_CORES = 8
K = 1024                 # padded contraction size (live rows)
KC = K // 128            # 8 k-chunks
NPC = 16384 // N_CORES   # 2048 output columns per core
NCH = NPC // 512         # 4 column chunks of 512
WS = 64.0                # fp8 weight scale (power of 2; x carries 1/WS)
F32 = mybir.dt.float32
F16 = mybir.dt.float16
F8 = mybir.dt.float8e3
U8 = mybir.dt.uint8

_BUILT = None            # cached nc so repeat calls reuse the compiled module
LAST_RESULTS = None      # BassKernelResults of the most recent run (for test.py)


def _build_bass():
    nc = bacc.Bacc(
        "TRN2", target_bir_lowering=False, debug=False, num_devices=N_CORES
    )
    # Pair 0 ships as two 512KB quarter-chunks (j=0, j=1) so its matmuls
    # (and epilogue) start earlier; pair 1 as one 1MB chunk.
    # w0[j]: [p, kc*512 + c] = W[kc*128+p, 512j + c] (x64, e3m4).
    # w1: [p, (j*KC+kc)*512 + c] = W[kc*128+p, 1024 + 512j + c].
    w0 = nc.dram_tensor("w0", [2, 128, KC * 512], F8, kind="ExternalInput").ap()
    w1 = nc.dram_tensor("w1", [128, 2 * KC * 512], F8, kind="ExternalInput").ap()
    # xs cols 0..63: stationary x blocks; cols 64..65: a [2,2] identity
    # for the K=2 bias matmul.
    xs = nc.dram_tensor("xs", [128, KC * 4 + 2], F16, kind="ExternalInput").ap()
    # bias rows packed [row(2), half(2)*512] f16 — moving operand of the
    # bias matmul.
    bh = nc.dram_tensor("bh", [2, 2 * 512], F16, kind="ExternalInput").ap()
    # aux packed [row(2), L|m2f x half(2)*512] f32: row j, col-block h
    # holds chunk 2h+j's values.
    aux = nc.dram_tensor("aux", [2, 2 * 1024], F32, kind="ExternalInput").ap()
    o = nc.dram_tensor("o", [NCH, 512], F32, kind="ExternalOutput").ap()

    with tile.TileContext(nc) as tc:
        with ExitStack() as ctx:
            small = ctx.enter_context(tc.tile_pool(name="small", bufs=1))
            wpool = ctx.enter_context(tc.tile_pool(name="wp", bufs=4))
            ppool = ctx.enter_context(tc.tile_pool(name="pp", bufs=1, space="PSUM"))
            scr = ctx.enter_context(tc.tile_pool(name="scr", bufs=1))

            # xs first on the sync queue (16KB; it gates the first matmul
            # and the scalar queue starts late behind the ACT table load),
            # then W as four 512KB quarter-chunks (h, j), FIFO — each
            # quarter releases 8 matmuls ~1.4us apart.
            xs_t = small.tile([128, KC * 4 + 2], F16, tag="xs")
            nc.sync.dma_start(xs_t[:], xs[:])
            wts = []
            for j in range(2):
                wt = wpool.tile([128, KC * 512], F8, tag="wq", name=f"wq{j}")
                nc.sync.dma_start(wt[:], w0[j])
                wts.append(wt)
            w1_t = wpool.tile([128, 2 * KC * 512], F8, tag="w1")
            nc.sync.dma_start(w1_t[:], w1[:])

            def wslice(half, j, kc):
                if half == 0:
                    return wts[j][:, kc * 512 : (kc + 1) * 512]
                return w1_t[:, (j * KC + kc) * 512 : (j * KC + kc + 1) * 512]

            # Small tensors on the scalar HWDGE queue: bh first (the bias
            # matmul opens each PSUM group), then aux.
            bh_t = small.tile([2, 2 * 512], F16, tag="bh")
            nc.scalar.dma_start(bh_t[:], bh[:])
            aux_t = small.tile([2, 2 * 1024], F32, tag="aux")
            nc.scalar.dma_start(aux_t[:], aux[:])
            l_t = aux_t[:, 0:1024]
            m2_t = aux_t[:, 1024:2048]

            # PE warm-up: ~3.4us of dummy matmuls during the dead preamble
            # window so the HAM un-throttles the PE clock before W arrives.
            wu = scr.tile([128, 448], F16, tag="wu")
            nc.gpsimd.memset(wu[:], 0.0)
            pw = ppool.tile([128, 512], F32, tag="pw")
            for _ in range(8):
                nc.tensor.matmul(
                    pw[0:1, 0:448], wu[:, 0:1], wu[:], start=True, stop=True
                )

            # Chunk pair {2h, 2h+1} -> PSUM bank h rows {0,1}: one
            # accumulation group per bank = K=2 bias matmul (identity
            # stationary x [2,512] bias rows, start=True) + 16 x-matmuls.
            # After the group closes, PSUM holds t = x'W + b directly.
            eye2 = xs_t[0:2, KC * 4 : KC * 4 + 2]
            pt0 = ppool.tile([128, 512], F32, tag="p0")
            pt1 = ppool.tile([128, 512], F32, tag="p1")
            pts = [pt0, pt1]
            for half in range(2):
                cs = slice(half * 512, (half + 1) * 512)
                pt = pts[half]
                nc.tensor.matmul(
                    pt[0:2, :], eye2, bh_t[:, cs], start=True, stop=False
                )
                for j in range(2):
                    for kc in range(KC):
                        blk = (kc * 2 + j) * 2
                        nc.tensor.matmul(
                            pt[0:2, :],
                            xs_t[:, blk : blk + 2],
                            wslice(half, j, kc),
                            start=False,
                            stop=(j == 1 and kc == KC - 1),
                        )

            # Epilogue per pair on contiguous [2,512], reading t from PSUM;
            # pair 0 overlaps pair 1's matmuls. y = max(t, L) * rc with
            # rc = 1/(1+|t*m2f|): the relu-select max runs on the DVE in
            # parallel with the ACT abs/+1 stage instead of after the mul.
            for half in range(2):
                cs = slice(half * 512, (half + 1) * 512)
                p2 = pts[half][0:2, :]
                tm = scr.tile([2, 512], F32, tag=f"tm{half}", name=f"tm{half}")
                mx = scr.tile([2, 512], F32, tag=f"mx{half}", name=f"mx{half}")
                at = scr.tile([2, 512], F32, tag=f"at{half}", name=f"at{half}")
                a1 = scr.tile([2, 512], F32, tag=f"a1{half}", name=f"a1{half}")
                rc = scr.tile([2, 512], F32, tag=f"rc{half}", name=f"rc{half}")
                sf = scr.tile([2, 512], F32, tag=f"sf{half}", name=f"sf{half}")
                nc.vector.tensor_mul(tm[:], p2, m2_t[:, cs])   # t * m2f
                nc.vector.tensor_max(mx[:], p2, l_t[:, cs])    # relu select
                nc.scalar.activation(                          # |t*m2f|
                    at[:], tm[:], mybir.ActivationFunctionType.Abs
                )
                nc.scalar.activation(                          # 1 + |t*m2f|
                    a1[:], at[:], mybir.ActivationFunctionType.Copy, bias=1.0
                )
                nc.vector.reciprocal_approx_fast(rc[:], a1[:])
                nc.vector.tensor_mul(sf[:], mx[:], rc[:])      # y
                nc.sync.dma_start(o[2 * half : 2 * half + 2], sf[:])

    nc.compile()
    return nc


def kernel(**inputs) -> np.ndarray:
    global _BUILT, LAST_RESULTS

    import ml_dtypes

    iv = np.asarray(inputs["input_values"], dtype=np.float32)
    W = np.asarray(inputs["weight_matrix"], dtype=np.float32)
    bias = np.asarray(inputs["biases"], dtype=np.float32)
    act = np.asarray(inputs["act_ids"])
    iidx = np.asarray(inputs["input_indices"]).astype(np.int64)
    oidx = np.asarray(inputs["output_indices"]).astype(np.int64)

    n = W.shape[0]
    # Dense neuron-state vector (duplicate indices: last write wins, matching
    # jax's .at[].set) and its index support.
    states = np.zeros(n, np.float32)
    states[iidx] = iv
    live = np.zeros(n, dtype=bool)
    live[iidx] = True
    support = np.flatnonzero(live)
    assert support.size <= K, "more than K live rows not supported"
    rows = np.zeros(K, np.int64)          # pad with row 0 (x=0 there => no-op)
    rows[: support.size] = support
    xvec = np.zeros(K, np.float32)
    xvec[: support.size] = states[support]

    Wq = (W[rows] * WS).astype(ml_dtypes.float8_e3m4)   # [K, n] live rows
    xh = (xvec / WS).astype(np.float16)
    xc = xh.reshape(KC, 128).T            # [128, KC]
    # Stationary blocks [128, (kc*2+j)*2 + m]: x chunk kc in column m==j;
    # trailing [2,2] identity for the bias matmul.
    xs_t = np.zeros((128, KC * 4 + 2), np.float16)
    for kc in range(KC):
        for j in range(2):
            xs_t[:, (kc * 2 + j) * 2 + j] = xc[:, kc]
    xs_t[0, KC * 4] = 1.0
    xs_t[1, KC * 4 + 1] = 1.0

    lsel = np.where(act == 1, 0.0, -3.0e38).astype(np.float32)
    m2f = (act == 2).astype(np.float32)

    in_maps = []
    for c in range(N_CORES):
        sl = slice(c * NPC, (c + 1) * NPC)
        # [kc, p, h, j, c] -> [h, p, j, kc, c]
        wc = Wq[:, sl].reshape(KC, 128, 2, 2, 512).transpose(2, 1, 3, 0, 4)

        def pack2(a):
            # [NCH*512] -> [row(2), half(2)*512]: packed[r, 512h+j] = chunk
            # (2h+r) col j, matching the aux SBUF layout.
            return a.reshape(2, 2, 512).transpose(1, 0, 2).reshape(2, 1024)

        in_maps.append(
            {
                "w0": np.ascontiguousarray(
                    wc[0].transpose(1, 0, 2, 3)     # [j, p, kc, c]
                ).reshape(2, 128, KC * 512),
                "w1": np.ascontiguousarray(wc[1]).reshape(128, 2 * KC * 512),
                "xs": xs_t,
                "bh": np.ascontiguousarray(
                    pack2(bias[sl].astype(np.float16))
                ),
                "aux": np.ascontiguousarray(
                    np.concatenate([pack2(lsel[sl]), pack2(m2f[sl])], axis=1)
                ),
            }
        )

    if _BUILT is None:
        _BUILT = _build_bass()
    LAST_RESULTS = run_bass_kernel_spmd(
        _BUILT, in_maps, core_ids=list(range(N_CORES))
    )
    full = np.concatenate(
        [LAST_RESULTS.results[c]["o"].reshape(-1) for c in range(N_CORES)]
    )
    return full[oidx].astype(np.float32)
